# revision 53
# baseline (speedup 1.0000x reference)
# Elman RNN (embedding gather -> recurrent tanh cell -> vocab projection)
# on 8 Trainium2 NeuronCores via Bass/Tile.
#
# Math (per time step t over T=256 steps, batch B=32, hid 1024):
#   H = tanh(W_xh[x_t] + H @ W_hh + b_h)        (serial recurrence)
#   Y_t = H @ W_hq + b_q                        (output projection)
# Returns (Y [T*B, 10000] f32, H_final [32, 1024] f32).
#
# Distribution: the recurrence is serial in t and cheap in FLOPs but must
# stream all of W_hh through the PE every step, so it is computed redundantly
# on every core in a layout-closed hid-major "W-stationary" form:
#   Z^T[m] = sum_k W_hh[k,m]^T @ H^T[k]  (+ X_t^T[m] via a selection matmul
#   that transposes the gathered embedding rows inside the PE array).
# b_h is folded into W_xh host-side (X~ = (W_xh + b_h)[ids], passed bf16).
# The projection (the bulk of the FLOPs and all output bytes) is sharded over
# the vocab dim: core c computes Y[:, c*1250:(c+1)*1250], fused into the step
# loop every 4 steps, reading H^T from an SBUF ring. b_q enters each PSUM
# group via a K=1 ones-row matmul. No collectives; the host concatenates the
# Y slices.
#
# NOTE on structure: TRN2 instructions encode at most ONE semaphore wait, so
# the program is arranged to keep every instruction at <=1 wait: single big
# tiles instead of rotating pool slots (no release churn), dep'd engine NOPs
# and dummy ldweights that absorb waits into each engine's clock, and DMAs
# ordered so ring-FIFO covers their hazards.

import os
import numpy as np
import ml_dtypes

import concourse.mybir as mybir
from concourse import bass
from concourse import tile
from concourse.tile_rust import add_dep_helper
from concourse.bass_utils import run_bass_kernel_spmd

F32 = mybir.dt.float32
BF16 = mybir.dt.bfloat16
I32 = mybir.dt.int32
TANH = mybir.ActivationFunctionType.Tanh
COPY = mybir.ActivationFunctionType.Copy

B = 32            # batch
T = 256           # time steps
HID = 1024        # hidden dim
VOCAB = 10000
OUT = 10000
NCORES = 8
OSL = OUT // NCORES   # out-dim slice per core (1250)
KC = HID // 128       # hid chunks of 128 (8)
W = 8                 # H^T SBUF ring depth (multiple of 4)
NSEG = [(0, 512), (512, 512), (1024, OSL - 1024)]  # psum bank segments of OSL


def _dep(a, b, sync=True):
    add_dep_helper(a.ins, b.ins, sync, "manual")


_SEM_OF_ENGINE = {"PE": "PE_44", "ACT": "Activation_44", "DVE": "DVE_44",
                  "PL": "Pool_44", "SP": "SP_44"}


def _strip_redundant_waits(nc):
    """TRN2 instructions encode at most one semaphore wait, but Tile's wait
    assigner is not transitively minimal. For instructions exceeding that
    budget, drop only provably-redundant waits: first own-engine-sem waits on
    compute instructions (same-engine completions are program-ordered), then
    waits already covered by an earlier instruction of the same wait stream.
    Instructions already within budget are left untouched.
    """
    waited: dict = {}
    own_map = {"PE": "PE_44", "Activation": "Activation_44",
               "DVE": "DVE_44", "Pool": "Pool_44", "SP": "SP_44"}
    for bb in nc.m.functions[0].blocks:
        for ins in bb.instructions:
            si = ins.sync_info
            if si is None or not si.on_wait:
                continue
            kind = type(ins).__name__
            if kind == "InstTileRelease":
                continue
            eng = str(ins.engine).split(".")[-1]
            is_dma = "DMA" in kind or "Dma" in kind
            if is_dma and str(getattr(ins, "queue", "")).startswith("qPool"):
                stream, own = ("SWDGE",), None
            else:
                stream = (eng,)
                own = None if is_dma else own_map.get(eng)

            waits = list(si.on_wait)

            def droppable_own(w):
                return (w.sync_type == "semaphore" and w.wait_reg is None
                        and own is not None and w.ant_name == own)

            def droppable_covered(w):
                return (w.sync_type == "semaphore" and w.wait_reg is None
                        and waited.get((stream, w.ant_name), -1) >= w.wait_value)

            for pred in (droppable_own, droppable_covered):
                i = 0
                while len(waits) > 1 and i < len(waits):
                    if pred(waits[i]):
                        del waits[i]
                    else:
                        i += 1

            for w in waits:
                if w.sync_type == "semaphore" and w.wait_reg is None:
                    key = (stream, w.ant_name)
                    if waited.get(key, -1) < w.wait_value:
                        waited[key] = w.wait_value
            if len(waits) != len(si.on_wait):
                si.on_wait = waits


def build_program(n_steps: int = T):
    assert n_steps % 4 == 0
    ngather = n_steps * B // 128          # 128-row gathers (t-major rows)
    nrchunk = n_steps * B // 128          # projection row chunks of 128

    nc = bass.Bass("TRN2", target_bir_lowering=False)

    ids_g = nc.dram_tensor("ids_g", [128, ngather], I32, kind="ExternalInput")
    state_t = nc.dram_tensor("state_t", [128, KC, B], BF16, kind="ExternalInput")
    w_xh = nc.dram_tensor("w_xh", [VOCAB, HID], BF16, kind="ExternalInput")
    w_hh = nc.dram_tensor("w_hh", [HID, HID], F32, kind="ExternalInput")
    w_hq = nc.dram_tensor("w_hq", [HID, OSL], F32, kind="ExternalInput")
    b_q = nc.dram_tensor("b_q", [1, OSL], F32, kind="ExternalInput")
    # 4 row-block selection matrices: ident[p, j, c] = (p == 32j + c)
    ident = nc.dram_tensor("ident", [128, 4, B], BF16, kind="ExternalInput")
    y_out = nc.dram_tensor("y_out", [n_steps * B, OSL], F32, kind="ExternalOutput")
    h_out = nc.dram_tensor("h_out", [B, HID], F32, kind="ExternalOutput")

    with tile.TileContext(nc) as tc:
        with tc.tile_pool(name="const", bufs=1) as cpool, \
             tc.tile_pool(name="yt", bufs=3) as yt_p, \
             tc.tile_pool(name="psrec", bufs=1, space="PSUM") as ps_p, \
             tc.tile_pool(name="psproj", bufs=1, space="PSUM") as yps_p:

            # --- constants ---
            hwdmas = []
            i4 = cpool.tile([128, 4, B], BF16)
            hwdmas.append(nc.sync.dma_start(out=i4[:], in_=ident[:]))
            idx_sb = cpool.tile([128, ngather], I32)
            hwdmas.append(nc.sync.dma_start(out=idx_sb[:], in_=ids_g[:]))

            # gathers come first on the SWDGE stream: gather 0 carries the
            # idx-load wait (fresh lane), later ones only their lane waits
            xall = cpool.tile([128, ngather, HID], BF16)
            gathers = []
            for g in range(ngather):
                gathers.append(nc.gpsimd.indirect_dma_start(
                    out=xall[:, g, :], out_offset=None,
                    in_=w_xh[:],
                    in_offset=bass.IndirectOffsetOnAxis(
                        ap=idx_sb[:, g:g + 1], axis=0),
                ))

            casts = []
            whh_sb = cpool.tile([128, KC, HID], BF16)   # [h, k, .] = W_hh[128k+h, .]
            for k in range(KC):
                casts.append(nc.gpsimd.dma_start(out=whh_sb[:, k, :],
                                    in_=w_hh[k * 128:(k + 1) * 128, :]))  # cast
            wq_sb = cpool.tile([128, KC, OSL], BF16)
            for k in range(KC):
                casts.append(nc.gpsimd.dma_start(out=wq_sb[:, k, :],
                                    in_=w_hq[k * 128:(k + 1) * 128, :]))  # cast
            pl_obs = casts[0]
            bq_f = cpool.tile([1, OSL], F32)
            hwdmas.append(nc.sync.dma_start(out=bq_f[:], in_=b_q[:]))
            bq_bf = cpool.tile([1, OSL], BF16)
            dve_ops = [nc.vector.tensor_copy(out=bq_bf[:], in_=bq_f[:])]
            ones_t = cpool.tile([1, 128], BF16)
            dve_ops.append(nc.vector.memset(ones_t[:], 1.0))

            # H^T ring (bf16, hid-major): h_all[p, m, slot, b]
            h_all = cpool.tile([128, KC, W, B], BF16)
            st_load = nc.sync.dma_start(out=h_all[:, :, W - 1, :], in_=state_t[:])
            hwdmas.append(st_load)

            hfin = cpool.tile([128, KC, B], F32)

            # prologue: dummy ldweights absorb input-DMA completions into the
            # PE clock (so real matmuls keep <=1 wait)
            pro = [nc.tensor.ldweights(weights=i4[:, 0, :]),
                   nc.tensor.ldweights(weights=h_all[:, 0, W - 1, :]),
                   nc.tensor.ldweights(weights=ones_t[0:1, :]),
                   nc.tensor.ldweights(weights=bq_bf[0:1, 0:128])]
            for k in range(KC):
                pro.append(nc.tensor.ldweights(weights=whh_sb[:, k, 0:128]))
                pro.append(nc.tensor.ldweights(weights=wq_sb[:, k, 0:128]))
            # absorbs the activation const-table init wait on ACT
            act_warm = cpool.tile([1, 4], F32, name="act_warm")
            aw = nc.scalar.activation(out=act_warm[:], in_=act_warm[:], func=TANH)

            prev_stop = [None, None]        # last stop-MM per half (psum WAW)
            slot_writer = [[st_load if s == W - 1 else None, None]
                           for s in range(W)]   # last writer per (slot, half)
            prev_ydma = [None, None, None]  # per yt buf index
            last_acts = []
            last_mmbs = []
            first_mm = None

            def emit_proj(r):
                # projection of steps 4r..4r+3 (called at step 4r+4 / at end)
                s0 = (4 * r) % W
                yps = yps_p.tile([128, OSL], F32, tag="yps", name=f"yps{r}")
                for (c0, cw) in NSEG:
                    for k in range(KC):
                        nc.tensor.matmul(
                            out=yps[:, c0:c0 + cw],
                            lhsT=h_all[:, k, s0:s0 + 4, :],
                            rhs=wq_sb[:, k, c0:c0 + cw],
                            start=(k == 0), stop=False)
                    mmb = nc.tensor.matmul(
                        out=yps[:, c0:c0 + cw],
                        lhsT=ones_t[0:1, :],
                        rhs=bq_bf[0:1, c0:c0 + cw],
                        start=False, stop=True)
                ybuf = r % 3
                # multi-wait ACT NoOp soaks the PE wait plus the lane waits of
                # all recent y stores; the eviction and store then stay at <=1
                for d in [mmb] + hwdmas[-12:]:
                    nopy = nc.scalar.nop(nofuse=True)
                    _dep(nopy, d)
                yt = yt_p.tile([128, OSL], F32, tag="yt", name=f"yt{r}")
                ev = nc.scalar.activation(out=yt[:], in_=yps[:], func=COPY)
                last_acts.append(ev)
                last_mmbs.append(mmb)
                del last_acts[:-2], last_mmbs[:-2]
                _dep(ev, nopy, sync=False)
                prev_ydma[ybuf] = nc.scalar.dma_start(
                    out=y_out[r * 128:(r + 1) * 128, :], in_=yt[:])
                hwdmas.append(prev_ydma[ybuf])

            for t in range(n_steps):
                # ---------------- fused projection (4 steps behind) ---------
                if t % 4 == 0 and t >= 4:
                    emit_proj(t // 4 - 1)

                # ---------------- recurrence step t ----------------
                g, j = divmod(t, 4)
                slot, pslot = t % W, (t - 1) % W
                xldw = None
                if j == 0:
                    # absorbs this gather's completion wait into the PE clock
                    xldw = nc.tensor.ldweights(weights=xall[0:B, g, 0:128])
                if slot_writer[pslot][1] is not None:
                    # multi-wait PE NoOp dep'd on the previous step's tanh:
                    # inherits (and thus soaks) the cross-engine Act wait AND
                    # the transitively-inherited PE/lane waits, so the
                    # following matmuls run waitless
                    dmm = nc.tensor.nop(nofuse=True)
                    _dep(dmm, slot_writer[pslot][1])
                else:
                    dmm = None
                for half in range(2):
                    ps = ps_p.tile([128, 4, B], F32, tag=f"ps{half}",
                                   name=f"ps{half}_{t}")
                    for mi in range(4):
                        m = half * 4 + mi
                        mm = nc.tensor.matmul(
                            out=ps[:, mi, :],
                            lhsT=whh_sb[:, 0, m * 128:(m + 1) * 128],
                            rhs=h_all[:, 0, pslot, :],
                            start=True, stop=False)
                        if dmm is not None:
                            _dep(mm, dmm, sync=False)
                        if xldw is not None:
                            _dep(mm, xldw, sync=False)
                        if first_mm is None:
                            first_mm = mm
                            for p in pro:
                                _dep(mm, p, sync=False)
                        # X_t^T[m] via PE transpose-by-selection
                        nc.tensor.matmul(
                            out=ps[:, mi, :],
                            lhsT=xall[:, g, m * 128:(m + 1) * 128],
                            rhs=i4[:, j, :],
                            start=False, stop=False)
                        for k in range(1, KC):
                            mmk = nc.tensor.matmul(
                                out=ps[:, mi, :],
                                lhsT=whh_sb[:, k, m * 128:(m + 1) * 128],
                                rhs=h_all[:, k, pslot, :],
                                start=False, stop=(k == KC - 1))
                        prev_stop[half] = mmk
                    # multi-wait ACT NoOp dep'd on this half's stop matmul:
                    # soaks the PE wait plus any inherited lane waits; the
                    # tanh itself then runs waitless
                    nopt = nc.scalar.nop(nofuse=True)
                    _dep(nopt, prev_stop[half])
                    th = nc.scalar.activation(
                        out=h_all[:, half * 4:(half + 1) * 4, slot, :],
                        in_=ps[:, :, :], func=TANH)
                    _dep(th, nopt, sync=False)
                    _dep(th, aw, sync=False)
                    slot_writer[slot][half] = th
                    if t == n_steps - 1:
                        nc.scalar.activation(
                            out=hfin[:, half * 4:(half + 1) * 4, :],
                            in_=ps[:, :, :], func=TANH)

            # final H (fp32, transposed scatter to [B, HID]; small, one-time)
            for d in hwdmas[-12:]:
                noph = nc.scalar.nop(nofuse=True)
                _dep(noph, d)
            for m in range(KC):
                hwdmas.append(nc.scalar.dma_start(
                    out=h_out[:, m * 128:(m + 1) * 128].rearrange("b p -> p b"),
                    in_=hfin[:, m, :]))

            emit_proj(n_steps // 4 - 1)

            # tail: consume every loose semaphore via single-dep SP nops so
            # the kernel-tail Drain needs no waits of its own
            tail_deps = ([prev_stop[0], prev_stop[1], pl_obs] + dve_ops
                         + [slot_writer[s][h] for s in range(W)
                            for h in range(2) if slot_writer[s][h] is not None]
                         + gathers[-8:] + hwdmas[-12:] + casts
                         + last_acts + last_mmbs)
            for d in tail_deps:
                tn = nc.sync.nop(nofuse=True)
                _dep(tn, d)

    _strip_redundant_waits(nc)
    return nc


_NC_CACHE = {}

# test-harness hooks (the grading path leaves these at defaults)
TRACE = bool(int(os.environ.get("RNN_TRACE", "0")))
LAST_RESULT = None


def _get_nc(n_steps: int = T):
    if n_steps not in _NC_CACHE:
        _NC_CACHE[n_steps] = build_program(n_steps)
    return _NC_CACHE[n_steps]


def kernel(inputs, state, W_xh, W_hh, b_h, W_hq, b_q):
    ids = np.asarray(inputs).astype(np.int64)
    state = np.asarray(state, dtype=np.float32)
    W_xh = np.asarray(W_xh, dtype=np.float32)
    W_hh = np.ascontiguousarray(np.asarray(W_hh, dtype=np.float32))
    b_h = np.asarray(b_h, dtype=np.float32)
    W_hq = np.asarray(W_hq, dtype=np.float32)
    b_q = np.asarray(b_q, dtype=np.float32)

    n_steps = ids.shape[1]
    assert ids.shape == (B, n_steps)
    ngather = n_steps * B // 128

    # fold b_h into the embedding table: X~ = (W_xh + b_h)[ids]; bf16 on device
    W_xh_eff = np.ascontiguousarray(
        (W_xh + b_h[None, :]).astype(ml_dtypes.bfloat16))

    # t-major flattened ids, arranged [partition, gather] for the index tile
    ids_tm = ids.T.reshape(-1)                       # row r = t*B + b
    ids_g = np.ascontiguousarray(
        ids_tm.reshape(ngather, 128).T.astype(np.int32))

    # hid-major transposed initial state: state_t[p, m, b] = state[b, 128m+p]
    state_t = np.ascontiguousarray(
        state.reshape(B, KC, 128).transpose(2, 1, 0).astype(ml_dtypes.bfloat16))

    # per-row-block selection matrices: ident[p, j, c] = (p == 32j + c)
    ident = np.zeros((128, 4, B), dtype=ml_dtypes.bfloat16)
    for jj in range(4):
        ident[jj * B:(jj + 1) * B, jj, :] = np.eye(B, dtype=ml_dtypes.bfloat16)
    ident = np.ascontiguousarray(ident)

    nc = _get_nc(n_steps)

    in_maps = []
    for c in range(NCORES):
        in_maps.append({
            "ids_g": ids_g,
            "state_t": state_t,
            "w_xh": W_xh_eff,
            "w_hh": W_hh,
            "ident": ident,
            "w_hq": np.ascontiguousarray(W_hq[:, c * OSL:(c + 1) * OSL]),
            "b_q": np.ascontiguousarray(
                b_q.reshape(1, OUT)[:, c * OSL:(c + 1) * OSL]),
        })

    res = run_bass_kernel_spmd(nc, in_maps, core_ids=list(range(NCORES)),
                               trace=TRACE)
    global LAST_RESULT
    LAST_RESULT = res
    outs = res.results if hasattr(res, "results") else res

    Y = np.concatenate([outs[c]["y_out"] for c in range(NCORES)], axis=1)
    H = outs[0]["h_out"]
    return Y, H
